# revision 2
# baseline (speedup 1.0000x reference)
"""CoStGcnBlock Trainium2 kernel.

Math (eval-mode, BN folded on host):
    Aw_s   = A_s * attn_s
    y      = relu(x + sum_s wg'_s @ (x . Aw_s) + b1eff)     (graph conv + BN1 + residual)
    out    = relu(x + conv_T(y, wt') + b2eff)               (9-tap temporal conv + BN2 + residual)

Device pipeline per (sample, 256-frame pair = 2 chunks of TC frames + 4-frame halos):
    1. DMA x fp32 -> SBUF [128=(chunk,c), (t,v)].
    2. Channel mix (fp32r): one matmul per (chunk, o-half, 512-col piece): lhsT columns =
       (s0,s1,s2,identity) x o32 -> u[(s|id, o32), (t, w32-padded)] in PSUM.
       Identity block carries the x-residual; a bias row written into PSUM carries b1eff.
    3. StreamTranspose 32x32 blocks (DVE) straight out of PSUM:
       u -> uT[(s|id, w32), (t, o32)] bf16 in SBUF.
    4. Graph mix: k=128 matmul with AwS (rows (s,w) = Aw_s[w,v], id rows = delta_wv + bias row)
       -> zT[v32, (t,o32)] in PSUM; 4 (chunk,half) slots col-tiled via tile_position.
    5. ACT relu -> yT bf16; StreamTranspose back -> y[(chunk,o64), (t, v32)].
    6. Temporal conv: 9 accumulated bf16 matmuls (k=c=64) per chunk, chunks col-tiled; rhs =
       shifted strided views of y; one extra fp32r identity matmul accumulates the x residual.
    7. ACT relu with per-partition bias b2eff -> fp32 out; DMA out.

Sharding: data-parallel over batch N: core k processes samples 4k..4k+3.
"""

import numpy as np
import ml_dtypes

N, C, O, T, V, K = 32, 64, 64, 1024, 25, 9
NCORES = 8
NS = N // NCORES
BN_EPS = 1e-5
HALO = 4
W32 = 32

BF16 = ml_dtypes.bfloat16


def _fold_weights(A, attn, wg, bg, bn1_gamma, bn1_beta, bn1_mean, bn1_var,
                  wt, bt, bn2_gamma, bn2_beta, bn2_mean, bn2_var):
    """Host-side constant folding. Returns device weight arrays."""
    f32 = np.float32
    Aw = (A * attn).astype(f32)                                   # (3, V, V)
    inv1 = (bn1_gamma / np.sqrt(bn1_var + BN_EPS)).astype(f32)    # (O,)
    b1eff = (bg.sum(0) * inv1 + bn1_beta - bn1_mean * inv1).astype(f32)
    wgp = (wg * inv1[None, :, None]).astype(f32)                  # (3, O, C)
    inv2 = (bn2_gamma / np.sqrt(bn2_var + BN_EPS)).astype(f32)
    wtp = (wt[:, :, :, 0] * inv2[:, None, None]).astype(f32)      # (O, C, K)
    b2eff = (bt * inv2 + bn2_beta - bn2_mean * inv2).astype(f32)

    # wch [128, 256] fp32: rows = c (duplicated 64..127); col 128*h + 32*b + j:
    #   b<3 -> wgp[b, 32h+j, c];  b=3 (identity) -> 1 if c == 32h+j.
    wch = np.zeros((64, 256), f32)
    for h in range(2):
        for b in range(3):
            wch[:, 128 * h + 32 * b:128 * h + 32 * b + 32] = wgp[b, 32 * h:32 * h + 32, :].T
        for j in range(32):
            wch[32 * h + j, 128 * h + 96 + j] = 1.0
    wch = np.concatenate([wch, wch], axis=0)

    # aws [128, 32]: rows 32b + w: b<3 -> Aw[b][w, v] (w<25); id block: delta_wv, row 31 = 1.
    aws = np.zeros((128, 32), f32)
    for b in range(3):
        aws[32 * b:32 * b + 25, :25] = Aw[b]
    for w in range(25):
        aws[96 + w, w] = 1.0
    aws[96 + 31, :25] = 1.0                                       # bias row
    # wt9 [128, 9*64]: rows = c (dup); col 64k + o = wtp[o, c, k]
    wt9 = np.zeros((64, 9 * 64), f32)
    for k in range(K):
        wt9[:, 64 * k:64 * k + 64] = wtp[:, :, k].T
    wt9 = np.concatenate([wt9, wt9], axis=0)

    b1t = np.zeros((128, 2), f32)
    for h in range(2):
        b1t[96:128, h] = b1eff[32 * h:32 * h + 32]
    b2t = np.concatenate([b2eff, b2eff]).reshape(128, 1).astype(f32)

    return {
        "wch": wch.astype(f32),
        "aws": aws.astype(f32),
        "wt9": wt9.astype(BF16),
        "i128": np.eye(128, dtype=f32),
        "z128": np.zeros((128, 128), f32),
        "b1t": b1t,
        "b2t": b2t,
    }


def _apv(tile_ap, p0, pn, free_off, free_dims):
    """Strided view of an SBUF/PSUM tile: partitions [p0, p0+pn), given free dims."""
    import concourse.bass as bass
    pitch = tile_ap.ap[0][0]
    return bass.AP(tensor=tile_ap.tensor, offset=p0 * pitch + free_off,
                   ap=[[pitch, pn]] + [list(d) for d in free_dims])


def _build_program(ns=NS, t_total=T, tc=128, u_piece=1024, u_bufs=2):
    """Build the Bass program for one core processing `ns` samples of `t_total` frames."""
    import concourse.bass as bass
    import concourse.mybir as mybir
    import concourse.tile as tile
    from concourse import bacc

    dt = mybir.dt
    AF = mybir.ActivationFunctionType

    th = tc + 2 * HALO            # frames per chunk incl halo
    f_x = th * V                  # x free size per chunk (packed v)
    f_u = th * W32                # u/uT/yT/y free size (padded v/w stride 32)
    f_out = tc * V
    npairs = t_total // (2 * tc)
    n_gchunks = (f_u + 511) // 512          # graph-mix psum chunks
    n_upieces = (f_u + u_piece - 1) // u_piece  # channel-mix psum pieces
    CT = 20                                  # conv out frames per chunk
    n_cchunks = (tc + CT - 1) // CT

    nc = bacc.Bacc("TRN2", target_bir_lowering=False, debug=False, num_devices=NCORES)

    xs = nc.dram_tensor("xs", [ns, C, t_total, V], dt.float32, kind="ExternalInput")
    wch_d = nc.dram_tensor("wch", [128, 256], dt.float32, kind="ExternalInput")
    aws_d = nc.dram_tensor("aws", [128, 32], dt.float32, kind="ExternalInput")
    wt9_d = nc.dram_tensor("wt9", [128, 9 * 64], dt.bfloat16, kind="ExternalInput")
    i128_d = nc.dram_tensor("i128", [128, 128], dt.float32, kind="ExternalInput")
    b1t_d = nc.dram_tensor("b1t", [128, 2], dt.float32, kind="ExternalInput")
    b2t_d = nc.dram_tensor("b2t", [128, 1], dt.float32, kind="ExternalInput")
    z128_d = nc.dram_tensor("z128", [128, 128], dt.float32, kind="ExternalInput")
    out_d = nc.dram_tensor("out", [ns, C, t_total, V], dt.float32, kind="ExternalOutput")

    ctv = C * t_total * V
    tv = t_total * V
    f32r = dt.float32r

    with tile.TileContext(nc) as tcx:
        import contextlib
        with contextlib.ExitStack() as ctx:
            const = ctx.enter_context(tcx.tile_pool(name="const", bufs=1))
            px = ctx.enter_context(tcx.tile_pool(name="px", bufs=5))
            put = ctx.enter_context(tcx.tile_pool(name="put", bufs=4))
            pyt = ctx.enter_context(tcx.tile_pool(name="pyt", bufs=2))
            py = ctx.enter_context(tcx.tile_pool(name="py", bufs=2))
            pout = ctx.enter_context(tcx.tile_pool(name="pout", bufs=2))
            ppu = ctx.enter_context(tcx.tile_pool(name="ppu", bufs=u_bufs, space="PSUM"))
            ppg = ctx.enter_context(tcx.tile_pool(name="ppg", bufs=2, space="PSUM"))
            ppc = ctx.enter_context(tcx.tile_pool(name="ppc", bufs=2, space="PSUM"))

            c_wch = const.tile([128, 256], f32r)
            nc.sync.dma_start(out=c_wch[:], in_=wch_d[:].bitcast(f32r))
            c_aws = const.tile([128, 32], dt.float32)
            nc.sync.dma_start(out=c_aws[:], in_=aws_d[:])
            c_wt9 = const.tile([128, 9 * 64], dt.bfloat16)
            nc.sync.dma_start(out=c_wt9[:], in_=wt9_d[:])
            c_i128 = const.tile([128, 128], f32r)
            nc.sync.dma_start(out=c_i128[:], in_=i128_d[:].bitcast(f32r))
            c_b1t = const.tile([128, 2], dt.float32)
            nc.sync.dma_start(out=c_b1t[:], in_=b1t_d[:])
            c_b2t = const.tile([128, 1], dt.float32)
            nc.sync.dma_start(out=c_b2t[:], in_=b2t_d[:])

            for n in range(ns):
                for p in range(npairs):
                    t0 = p * 2 * tc
                    x_sb = px.tile([128, f_x + 8], f32r, tag="x")

                    first = p == 0
                    last = p == npairs - 1
                    if not first and not last:
                        # +8 tail guard reads the next frames (real, finite data)
                        in_ap = bass.AP(tensor=xs, offset=n * ctv + (t0 - HALO) * V,
                                        ap=[[tc * V, 2], [tv, C], [1, f_x + 8]])
                        nc.sync.dma_start(out=x_sb[:, 0:f_x + 8], in_=in_ap.bitcast(f32r))
                    else:
                        for ch in range(2):
                            tlo = t0 + tc * ch - HALO
                            thi = tlo + th
                            vlo, vhi = max(tlo, 0), min(thi + 1, t_total)
                            doff = (vlo - tlo) * V
                            dlen = (vhi - vlo) * V
                            dlen = min(dlen, f_x + 8 - doff)
                            if doff > 0:   # zero-fill left gap
                                nc.sync.dma_start(
                                    out=_apv(x_sb[:], 64 * ch, 64, 0, [[1, doff]]),
                                    in_=_apv(z128_d.ap(), 0, 64, 0,
                                             [[0, (doff + 127) // 128],
                                              [1, min(doff, 128)]]).bitcast(f32r)
                                    if doff > 128 else
                                    _apv(z128_d.ap(), 0, 64, 0, [[1, doff]]).bitcast(f32r))
                            if doff + dlen < f_x + 8:   # zero-fill right gap + tail
                                glen = f_x + 8 - (doff + dlen)
                                nc.sync.dma_start(
                                    out=_apv(x_sb[:], 64 * ch, 64, doff + dlen, [[1, glen]]),
                                    in_=_apv(z128_d.ap(), 0, 64, 0,
                                             [[0, (glen + 127) // 128],
                                              [1, min(glen, 128)]]).bitcast(f32r))
                            in_ap = bass.AP(tensor=xs, offset=n * ctv + vlo * V,
                                            ap=[[tv, C], [1, dlen]])
                            out_ap = _apv(x_sb[:], 64 * ch, 64, doff, [[1, dlen]])
                            nc.sync.dma_start(out=out_ap, in_=in_ap.bitcast(f32r))

                    # ---- channel mix (fp32r) + transpose to uT ----
                    uT = {}
                    for ch in range(2):
                        for h in range(2):
                            ut = put.tile([128, f_u], dt.float32, tag="ut")
                            uT[(ch, h)] = ut
                            tpp = u_piece // W32   # frames per psum piece
                            for gp in range(n_upieces):
                                pcols = min(u_piece, f_u - gp * u_piece)
                                u_ps = ppu.tile([128, u_piece], dt.float32, tag="upsum")
                                for j in range((pcols + 511) // 512):
                                    nt = min(16, th - (gp * tpp + j * 16))
                                    cols = nt * W32
                                    rhs = _apv(x_sb[:], 64 * ch, 64,
                                               (gp * tpp + j * 16) * V,
                                               [[V, nt], [1, W32]])
                                    nc.tensor.matmul(
                                        u_ps[:, j * 512:j * 512 + cols],
                                        c_wch[64 * ch:64 * ch + 64,
                                              128 * h:128 * h + 128],
                                        rhs, start=True, stop=True,
                                        tile_position=(64 * ch, 0))
                                # bias row into PSUM: u[(id,o), (t, w=31)] = b1eff[o]
                                ntp = pcols // W32
                                bias_out = _apv(u_ps[:], 96, 32, 31, [[W32, ntp]])
                                bias_in = _apv(c_b1t[:], 96, 32, h, [[0, ntp]])
                                nc.scalar.activation(out=bias_out, in_=bias_in,
                                                     func=AF.Copy, bias=0.0, scale=1.0)
                                nc.vector.transpose(
                                    out=ut[:, gp * u_piece:gp * u_piece + pcols],
                                    in_=u_ps[:, 0:pcols])

                    # ---- graph mix (+ residual + bias via id rows) ----
                    yT = pyt.tile([128, f_u], dt.bfloat16, tag="yt")
                    slots = [(0, 0), (0, 1), (1, 0), (1, 1)]
                    for g in range(n_gchunks):
                        cols = min(512, f_u - g * 512)
                        g_ps = ppg.tile([128, 512], dt.float32, tag="gpsum")
                        for j, key in enumerate(slots):
                            nc.tensor.matmul(
                                g_ps[32 * j:32 * j + 32, 0:cols],
                                c_aws[:, 0:32],
                                uT[key][:, g * 512:g * 512 + cols],
                                start=True, stop=True, tile_position=(0, 32 * j))
                        nc.scalar.activation(out=yT[:, g * 512:g * 512 + cols],
                                             in_=g_ps[:, 0:cols], func=AF.Relu,
                                             bias=0.0, scale=1.0)

                    # ---- transpose back yT -> y [(ch, o64), (t, v32)] ----
                    y = py.tile([128, f_u], dt.bfloat16, tag="y")
                    nc.vector.transpose(out=y[:], in_=yT[:])
                    if first:
                        nc.vector.memset(y[0:64, 0:HALO * W32], 0.0)
                    if last:
                        nc.vector.memset(y[64:128, (th - HALO) * W32:f_u], 0.0)

                    # ---- temporal conv + x residual + BN2 + relu ----
                    out_sb = pout.tile([128, f_out], dt.float32, tag="o")
                    for g3 in range(n_cchunks):
                        nt = min(CT, tc - g3 * CT)
                        cols = nt * V
                        c_ps = ppc.tile([128, 500], dt.float32, tag="cpsum")
                        for ch in range(2):
                            for k in range(K):
                                rhs = _apv(y[:], 64 * ch, 64, (g3 * CT + k) * W32,
                                           [[W32, nt], [1, V]])
                                nc.tensor.matmul(
                                    c_ps[64 * ch:64 * ch + 64, 0:cols],
                                    c_wt9[64 * ch:64 * ch + 64, 64 * k:64 * k + 64],
                                    rhs, start=(k == 0), stop=False,
                                    tile_position=(64 * ch, 64 * ch))
                        x_res = _apv(x_sb[:], 0, 128, (HALO + g3 * CT) * V,
                                     [[1, cols]])
                        nc.tensor.matmul(c_ps[:, 0:cols],
                                         c_i128[:], x_res,
                                         start=False, stop=True,
                                         tile_position=(0, 0),
                                         skip_group_check=True)
                        nc.scalar.activation(out=out_sb[:, g3 * CT * V:g3 * CT * V + cols],
                                             in_=c_ps[:, 0:cols], func=AF.Relu,
                                             bias=c_b2t[:, 0:1], scale=1.0)

                    out_ap = bass.AP(tensor=out_d, offset=n * ctv + t0 * V,
                                     ap=[[tc * V, 2], [tv, C], [1, f_out]])
                    nc.sync.dma_start(out=out_ap, in_=out_sb[:])

    nc.finalize()
    return nc


_CACHE = {}


def _get_program(ns, t_total, tc, **kw):
    key = (ns, t_total, tc, tuple(sorted(kw.items())))
    if key not in _CACHE:
        _CACHE[key] = _build_program(ns, t_total, tc, **kw)
    return _CACHE[key]


def _program_and_maps(inputs):
    x = np.asarray(inputs["x"], dtype=np.float32)
    w = _fold_weights(
        np.asarray(inputs["A"]), np.asarray(inputs["attn"]),
        np.asarray(inputs["wg"]), np.asarray(inputs["bg"]),
        np.asarray(inputs["bn1_gamma"]), np.asarray(inputs["bn1_beta"]),
        np.asarray(inputs["bn1_mean"]), np.asarray(inputs["bn1_var"]),
        np.asarray(inputs["wt"]), np.asarray(inputs["bt"]),
        np.asarray(inputs["bn2_gamma"]), np.asarray(inputs["bn2_beta"]),
        np.asarray(inputs["bn2_mean"]), np.asarray(inputs["bn2_var"]))

    nc = _get_program(NS, T, 128)
    in_maps = []
    for k in range(NCORES):
        m = {"xs": np.ascontiguousarray(x[NS * k:NS * k + NS])}
        m.update(w)
        in_maps.append(m)
    return nc, in_maps


def kernel(**inputs):
    from concourse.bass_utils import run_bass_kernel_spmd

    nc, in_maps = _program_and_maps(inputs)
    res = run_bass_kernel_spmd(nc, in_maps, core_ids=list(range(NCORES)))
    return np.concatenate([r["out"] for r in res.results], axis=0)



# revision 12
# speedup vs baseline: 1.5536x; 1.5536x over previous
"""CoStGcnBlock Trainium2 kernel (v2, all-bf16 dataflow).

Math (eval-mode, BN folded on host):
    Aw_s   = A_s * attn_s
    y      = relu(x + sum_s wg'_s @ (x . Aw_s) + b1eff)     (graph conv + BN1 + residual)
    out    = relu(x + conv_T(y, wt') + b2eff)               (9-tap temporal conv + BN2 + residual)

Device pipeline per (sample, 256-frame pair = 2 chunks of 128 frames + 4-frame halos):
    1. DMA x bf16 -> SBUF [128=(chunk,c), (t,v25)].
    2. Channel mix (bf16): per (chunk, o-half): matmuls lhsT = (s0,s1,s2,identity) x o32
       -> u[(s|id, o32), (t, w32-padded)] in PSUM (1024-col pieces).
    3. StreamTranspose 32x32 blocks (DVE) out of PSUM with bf16 cast:
       u -> uT[(s|id, w32), (t, o32)] bf16 in SBUF.
    4. Graph mix: k=128 matmul with aws (rows (s,w) = Aw_s[w,v], id rows = delta_wv)
       -> zT[v32, (t,o32)] in PSUM; 4 (chunk,half) slots col-tiled via tile_position.
    5. StreamTranspose back (cast bf16) -> y_pre[(chunk,o64), (t, v32)];
       ACT relu with per-partition bias b1eff -> y bf16.
    6. Temporal conv: tap-outer accumulated bf16 matmuls, lhsT = per-tap block-diag
       [128,128] covering both chunks; bf16 identity matmul adds the x residual.
    7. ACT relu with per-partition bias b2eff -> fp32 out; DMA out.

Sharding: data-parallel over batch N: core k processes samples 4k..4k+3.
"""

import numpy as np
import ml_dtypes

N, C, O, T, V, K = 32, 64, 64, 1024, 25, 9
NCORES = 8
NS = N // NCORES
BN_EPS = 1e-5
HALO = 4
W32 = 32

BF16 = ml_dtypes.bfloat16


def _fold_weights(A, attn, wg, bg, bn1_gamma, bn1_beta, bn1_mean, bn1_var,
                  wt, bt, bn2_gamma, bn2_beta, bn2_mean, bn2_var):
    """Host-side constant folding. Returns device weight arrays."""
    f32 = np.float32
    Aw = (A * attn).astype(f32)                                   # (3, V, V)
    inv1 = (bn1_gamma / np.sqrt(bn1_var + BN_EPS)).astype(f32)    # (O,)
    b1eff = (bg.sum(0) * inv1 + bn1_beta - bn1_mean * inv1).astype(f32)
    wgp = (wg * inv1[None, :, None]).astype(f32)                  # (3, O, C)
    inv2 = (bn2_gamma / np.sqrt(bn2_var + BN_EPS)).astype(f32)
    wtp = (wt[:, :, :, 0] * inv2[:, None, None]).astype(f32)      # (O, C, K)
    b2eff = (bt * inv2 + bn2_beta - bn2_mean * inv2).astype(f32)

    # wch [128, 256]: rows = c (duplicated 64..127); col 128*h + 32*b + j:
    #   b<3 -> wgp[b, 32h+j, c];  b=3 (identity) -> 1 if c == 32h+j.
    wch = np.zeros((64, 256), f32)
    for h in range(2):
        for b in range(3):
            wch[:, 128 * h + 32 * b:128 * h + 32 * b + 32] = wgp[b, 32 * h:32 * h + 32, :].T
        for j in range(32):
            wch[32 * h + j, 128 * h + 96 + j] = 1.0
    wch = np.concatenate([wch, wch], axis=0)

    # aws [128, 32]: rows 32b + w: b<3 -> Aw[b][w, v] (w<25); id block: delta_wv.
    aws = np.zeros((128, 32), f32)
    for b in range(3):
        aws[32 * b:32 * b + 25, :25] = Aw[b]
    for w in range(25):
        aws[96 + w, w] = 1.0

    # wtbd [128, 9*128]: per-tap block-diag: both (chunk) 64-blocks = wtp[:, :, k].T
    wtbd = np.zeros((128, 9 * 128), f32)
    for k in range(K):
        blk = wtp[:, :, k].T                                      # [c, o]
        wtbd[0:64, 128 * k:128 * k + 64] = blk
        wtbd[64:128, 128 * k + 64:128 * k + 128] = blk

    b1t = np.concatenate([b1eff, b1eff]).reshape(128, 1).astype(f32)
    b2t = np.concatenate([b2eff, b2eff]).reshape(128, 1).astype(f32)

    return {
        "wch": wch.astype(BF16),
        "aws": aws,
        "wtbd": wtbd.astype(BF16),
        "i128": np.eye(128, dtype=f32).astype(BF16),
        "b1t": b1t,
        "b2t": b2t,
    }


def _apv(tile_ap, p0, pn, free_off, free_dims):
    """Strided view of an SBUF/PSUM tile: partitions [p0, p0+pn), given free dims."""
    import concourse.bass as bass
    pitch = tile_ap.ap[0][0]
    return bass.AP(tensor=tile_ap.tensor, offset=p0 * pitch + free_off,
                   ap=[[pitch, pn]] + [list(d) for d in free_dims])


def _build_program(ns=NS, t_total=T, tc=128):
    """Build the Bass program for one core processing `ns` samples of `t_total` frames."""
    import concourse.bass as bass
    import concourse.mybir as mybir
    import concourse.tile as tile
    from concourse import bacc

    dt = mybir.dt
    AF = mybir.ActivationFunctionType

    th = tc + 2 * HALO            # frames per chunk incl halo (136)
    f_x = th * V                  # x free size per chunk, packed v (3400)
    f_u = th * W32                # u/uT/y free size, padded v/w stride 32 (4352)
    f_out = tc * V                # 3200
    npairs = t_total // (2 * tc)  # 4
    CT = 20                       # conv out frames per psum group
    n_cgroups = (tc + CT - 1) // CT   # 7 (6x20 + 8)
    UP = 1024                     # channel-mix psum piece cols
    n_upieces = (f_u + UP - 1) // UP  # 5 (4x1024 + 256)
    n_gchunks = (f_u + 511) // 512    # 9 (8x512 + 256)

    nc = bacc.Bacc("TRN2", target_bir_lowering=False, debug=False, num_devices=NCORES)
    f32r = dt.float32r

    xs = nc.dram_tensor("xs", [ns, C, t_total, V], dt.bfloat16, kind="ExternalInput")
    wch_d = nc.dram_tensor("wch", [128, 256], dt.bfloat16, kind="ExternalInput")
    aws_d = nc.dram_tensor("aws", [128, 32], dt.float32, kind="ExternalInput")
    wtbd_d = nc.dram_tensor("wtbd", [128, 9 * 128], dt.bfloat16, kind="ExternalInput")
    i128_d = nc.dram_tensor("i128", [128, 128], dt.bfloat16, kind="ExternalInput")
    b1t_d = nc.dram_tensor("b1t", [128, 1], dt.float32, kind="ExternalInput")
    b2t_d = nc.dram_tensor("b2t", [128, 1], dt.float32, kind="ExternalInput")
    out_d = nc.dram_tensor("out", [ns, C, t_total, V], dt.float32, kind="ExternalOutput")

    ctv = C * t_total * V
    tv = t_total * V

    with tile.TileContext(nc) as tcx:
        import contextlib
        with contextlib.ExitStack() as ctx:
            const = ctx.enter_context(tcx.tile_pool(name="const", bufs=1))
            px = ctx.enter_context(tcx.tile_pool(name="px", bufs=3))
            put = ctx.enter_context(tcx.tile_pool(name="put", bufs=6))
            pyp = ctx.enter_context(tcx.tile_pool(name="pyp", bufs=2))
            py = ctx.enter_context(tcx.tile_pool(name="py", bufs=2))
            pout = ctx.enter_context(tcx.tile_pool(name="pout", bufs=2))
            ppu = ctx.enter_context(tcx.tile_pool(name="ppu", bufs=2, space="PSUM"))
            ppg = ctx.enter_context(tcx.tile_pool(name="ppg", bufs=2, space="PSUM"))
            ppc = ctx.enter_context(tcx.tile_pool(name="ppc", bufs=2, space="PSUM"))

            c_wch = const.tile([128, 256], dt.bfloat16)
            nc.sync.dma_start(out=c_wch[:], in_=wch_d[:])
            c_aws = const.tile([128, 32], dt.float32)
            nc.sync.dma_start(out=c_aws[:], in_=aws_d[:])
            c_wtbd = const.tile([128, 9 * 128], dt.bfloat16)
            nc.sync.dma_start(out=c_wtbd[:], in_=wtbd_d[:])
            c_i128 = const.tile([128, 128], dt.bfloat16)
            nc.sync.dma_start(out=c_i128[:], in_=i128_d[:])
            c_b1t = const.tile([128, 1], dt.float32)
            nc.sync.dma_start(out=c_b1t[:], in_=b1t_d[:])
            c_b2t = const.tile([128, 1], dt.float32)
            nc.sync.dma_start(out=c_b2t[:], in_=b2t_d[:])

            for n in range(ns):
                for p in range(npairs):
                    t0 = p * 2 * tc
                    first = p == 0
                    last = p == npairs - 1
                    x_sb = px.tile([128, f_x + 8 * V], dt.bfloat16, tag="x")

                    if not first and not last:
                        # interior: one DMA, incl 8-frame tail guard (real data)
                        in_ap = bass.AP(tensor=xs, offset=n * ctv + (t0 - HALO) * V,
                                        ap=[[tc * V, 2], [tv, C], [1, f_x + 8 * V]])
                        nc.sync.dma_start(out=x_sb[:], in_=in_ap)
                    else:
                        for ch in range(2):
                            tlo = t0 + tc * ch - HALO
                            thi = tlo + th + 8
                            vlo, vhi = max(tlo, 0), min(thi, t_total)
                            doff = (vlo - tlo) * V
                            dlen = (vhi - vlo) * V
                            if doff > 0:
                                nc.vector.memset(
                                    _apv(x_sb[:], 64 * ch, 64, 0, [[1, doff]]), 0.0)
                            if doff + dlen < f_x + 8 * V:
                                glen = f_x + 8 * V - (doff + dlen)
                                nc.vector.memset(
                                    _apv(x_sb[:], 64 * ch, 64, doff + dlen,
                                         [[1, glen]]), 0.0)
                            in_ap = bass.AP(tensor=xs, offset=n * ctv + vlo * V,
                                            ap=[[tv, C], [1, dlen]])
                            nc.sync.dma_start(
                                out=_apv(x_sb[:], 64 * ch, 64, doff, [[1, dlen]]),
                                in_=in_ap)

                    # ---- channel mix (bf16) + transpose to uT (fp32 -> f32r) ----
                    uT = {}
                    for ch in range(2):
                        for h in range(2):
                            ut = put.tile([128, f_u], dt.float32, tag="ut")
                            uT[(ch, h)] = ut
                            for gp in range(n_upieces):
                                pcols = min(UP, f_u - gp * UP)
                                u_ps = ppu.tile([128, UP], dt.float32, tag="upsum")
                                for j in range((pcols + 511) // 512):
                                    nt = min(16, th - (gp * (UP // W32) + j * 16))
                                    cols = nt * W32
                                    rhs = _apv(x_sb[:], 64 * ch, 64,
                                               (gp * (UP // W32) + j * 16) * V,
                                               [[V, nt], [1, W32]])
                                    nc.tensor.matmul(
                                        u_ps[:, j * 512:j * 512 + cols],
                                        c_wch[64 * ch:64 * ch + 64,
                                              128 * h:128 * h + 128],
                                        rhs, start=True, stop=True,
                                        tile_position=(64 * ch, 0))
                                nc.vector.transpose(
                                    out=ut[:, gp * UP:gp * UP + pcols],
                                    in_=u_ps[:, 0:pcols])

                    # ---- graph mix (k=128 f32r; x-residual via id rows) ----
                    # back-transpose into small fp32 staging, then ACT
                    # relu + per-partition b1eff -> y bf16
                    y = py.tile([128, f_u], dt.bfloat16, tag="y")
                    slots = [(0, 0), (0, 1), (1, 0), (1, 1)]
                    for g0 in range(0, n_gchunks, 2):
                        yp = pyp.tile([128, 1024], dt.float32, tag="yp")
                        pc0 = min(1024, f_u - g0 * 512)
                        for g in (g0, g0 + 1):
                            if g >= n_gchunks:
                                continue
                            cols = min(512, f_u - g * 512)
                            g_ps = ppg.tile([128, 512], dt.float32, tag="gpsum")
                            for j, key in enumerate(slots):
                                nc.tensor.matmul(
                                    g_ps[32 * j:32 * j + 32, 0:cols],
                                    c_aws[:, 0:32],
                                    uT[key][:, g * 512:g * 512 + cols],
                                    start=True, stop=True,
                                    tile_position=(0, 32 * j))
                            nc.vector.transpose(
                                out=yp[:, (g - g0) * 512:(g - g0) * 512 + cols],
                                in_=g_ps[:, 0:cols])
                        nc.scalar.activation(out=y[:, g0 * 512:g0 * 512 + pc0],
                                             in_=yp[:, 0:pc0], func=AF.Relu,
                                             bias=c_b1t[:, 0:1], scale=1.0)
                    if first:
                        nc.vector.memset(y[0:64, 0:HALO * W32], 0.0)
                    if last:
                        nc.vector.memset(y[64:128, (th - HALO) * W32:f_u], 0.0)

                    # ---- temporal conv (tap-outer, block-diag) + x residual ----
                    out_sb = pout.tile([128, f_out], dt.float32, tag="o")
                    for g0 in range(0, n_cgroups, 2):
                        gs = [g for g in (g0, g0 + 1) if g < n_cgroups]
                        cps = {}
                        for g in gs:
                            c_ps = ppc.tile([128, CT * V], dt.float32, tag="cpsum")
                            cps[g] = c_ps
                        # x residual (identity matmul, start of accumulation)
                        for g in gs:
                            nt = min(CT, tc - g * CT)
                            cols = nt * V
                            x_res = _apv(x_sb[:], 0, 128, (HALO + g * CT) * V,
                                         [[1, cols]])
                            nc.tensor.matmul(cps[g][:, 0:cols], c_i128[:], x_res,
                                             start=True, stop=False,
                                             tile_position=(0, 0))
                        for k in range(K):
                            for g in gs:
                                nt = min(CT, tc - g * CT)
                                cols = nt * V
                                rhs = _apv(y[:], 0, 128, (g * CT + k) * W32,
                                           [[W32, nt], [1, V]])
                                nc.tensor.matmul(
                                    cps[g][:, 0:cols],
                                    c_wtbd[:, 128 * k:128 * k + 128],
                                    rhs, start=False, stop=(k == K - 1),
                                    tile_position=(0, 0), skip_group_check=True)
                        for g in gs:
                            nt = min(CT, tc - g * CT)
                            cols = nt * V
                            nc.scalar.activation(
                                out=out_sb[:, g * CT * V:g * CT * V + cols],
                                in_=cps[g][:, 0:cols], func=AF.Relu,
                                bias=c_b2t[:, 0:1], scale=1.0)

                    out_ap = bass.AP(tensor=out_d, offset=n * ctv + t0 * V,
                                     ap=[[tc * V, 2], [tv, C], [1, f_out]])
                    nc.sync.dma_start(out=out_ap, in_=out_sb[:])

    nc.finalize()
    return nc


_CACHE = {}


def _get_program(ns, t_total, tc, **kw):
    key = (ns, t_total, tc, tuple(sorted(kw.items())))
    if key not in _CACHE:
        _CACHE[key] = _build_program(ns, t_total, tc, **kw)
    return _CACHE[key]


def _program_and_maps(inputs):
    x = np.asarray(inputs["x"], dtype=np.float32)
    w = _fold_weights(
        np.asarray(inputs["A"]), np.asarray(inputs["attn"]),
        np.asarray(inputs["wg"]), np.asarray(inputs["bg"]),
        np.asarray(inputs["bn1_gamma"]), np.asarray(inputs["bn1_beta"]),
        np.asarray(inputs["bn1_mean"]), np.asarray(inputs["bn1_var"]),
        np.asarray(inputs["wt"]), np.asarray(inputs["bt"]),
        np.asarray(inputs["bn2_gamma"]), np.asarray(inputs["bn2_beta"]),
        np.asarray(inputs["bn2_mean"]), np.asarray(inputs["bn2_var"]))

    xb = x.astype(BF16)
    nc = _get_program(NS, T, 128)
    in_maps = []
    for k in range(NCORES):
        m = {"xs": np.ascontiguousarray(xb[NS * k:NS * k + NS])}
        m.update(w)
        in_maps.append(m)
    return nc, in_maps


def kernel(**inputs):
    from concourse.bass_utils import run_bass_kernel_spmd

    nc, in_maps = _program_and_maps(inputs)
    res = run_bass_kernel_spmd(nc, in_maps, core_ids=list(range(NCORES)))
    return np.concatenate([r["out"] for r in res.results], axis=0)
